# revision 21
# baseline (speedup 1.0000x reference)
"""Multi-head self-attention (B=2, T=2048, C=1024, H=16) on 8 trn2 NeuronCores.

Sharding: tensor-parallel over heads x data-parallel over batch.
Core c handles batch b = c//4 and the 4 heads [4g, 4g+4) where g = c%4.

v2 design (bf16 operands, fp32 PSUM accumulation):
  - All matmul operands bf16 (x, W, qkT, v_aug, E); halves DMA bytes and
    weight-load time (FWL), PSUM accumulate stays fp32.
  - DMA order: biases, qk-pair0 weights, wv, x by 512-token chunks,
    qk-pair1 weights.  Projection chunks are interleaved into the
    attention step stream so PE never waits on DMA after the first chunk.
  - qkT proj in transposed layout [d, tokens]; q pre-scaled 1/8 on host.
    v proj in natural layout [token, d] stored ones-augmented
    v_aug[t, 65h+d], col 65h+64 = 1.0 (gives softmax denominator via PV).
  - 128 attention steps (unit u = head-pair pr x q-tile qt, k-tile kt):
    S^T pair: two K=64 matmuls (array row-halves) into one 2-bank PSUM
    tile [128, 1024]; one wide exp on ScalarE -> E tile (SBUF, bf16).
    P.V lags LAG steps: po[h2] [65, 512] accumulates v_aug.T @ E over 16
    k-tiles (row 64 = softmax denominator).
  - Finalize per unit (no PE work): DVE reciprocal of denominator row,
    GpSimd partition-broadcast, DVE multiply PSUM*recip -> y_sb [64, T]
    per head (transposed layout), DMA out yT [256, T]; host transposes.
"""

import numpy as np
import ml_dtypes

import concourse.bass as bass
import concourse.mybir as mybir
import concourse.tile as tile
from concourse import bacc
from concourse.bass_utils import run_bass_kernel_spmd

N_CORES = 8
B, T, C = 2, 2048, 1024
D = 64            # head dim
HPC = 4           # heads per core
NT_C = C // 128   # 8 contraction tiles
NT_T = T // 128   # 16 token (k) tiles
NQ = T // 512     # 4 q-column tiles
LAG = 4           # PV runs this many steps behind S/exp
F32 = mybir.dt.float32
BF16 = mybir.dt.bfloat16
I16 = mybir.dt.int16
NPBF16 = ml_dtypes.bfloat16
# bf16 Schraudolph exp: bitcast_bf16(int16(x * 128*log2e + B)), |rel err| <~ 3.5%
SCH_A = float(128.0 / np.log(2.0))
SCH_B = 16249.95

_BUILT = None
LAST_RESULT = None


def _build():
    nc = bacc.Bacc(None, target_bir_lowering=False)

    xT = nc.dram_tensor("xT", [C, T], BF16, kind="ExternalInput")
    # col blocks of 128: [q-pair0 | k-pair0 | q-pair1 | k-pair1]
    wqk = nc.dram_tensor("wqk", [C, 512], BF16, kind="ExternalInput")
    wv = nc.dram_tensor("wv", [C, 256], BF16, kind="ExternalInput")
    # rows: [q-pair0, k-pair0, q-pair1, k-pair1] biases
    bqk = nc.dram_tensor("bqk", [4, 128], F32, kind="ExternalInput")
    bv = nc.dram_tensor("bv", [1, 256], F32, kind="ExternalInput")
    yT = nc.dram_tensor("yT", [256, T], F32, kind="ExternalOutput")

    OTQ = (0, 2)   # q col-block index per pair
    OTK = (1, 3)   # k col-block index per pair

    with tile.TileContext(nc) as tc:
        with tc.tile_pool(name="persist", bufs=1) as sb:
            bqk_sb = sb.tile([128, 4], F32)
            bv_sb = sb.tile([1, 256], F32)
            bv_bc = sb.tile([128, 256], F32)
            ones_col = sb.tile([128, 1], BF16)
            warm = sb.tile([128, 1], F32)
            qkT = sb.tile([128, 4, T], BF16)
            v_aug = sb.tile([128, NT_T, HPC * 65], BF16)
            # y in transposed layout: per local head h, [64 d, T tokens]
            y_sb = sb.tile([64, HPC, T], F32)

            with tc.tile_pool(name="io", bufs=1) as io:
                xT_sb = io.tile([128, NT_C, T], BF16)
                wqk_sb = io.tile([128, NT_C, 512], BF16)
                wv_sb = io.tile([128, NT_C, 256], BF16)

                # preload the exp table set on ScalarE (~2.7us) before it
                # is needed; ones_col is memset below, warm is scratch
                nc.vector.memset(ones_col[:, :], 1.0)
                nc.scalar.activation(warm[:, :], ones_col[:, :],
                                     mybir.ActivationFunctionType.Exp)
                nc.vector.tensor_copy(
                    v_aug.rearrange("p k (h e) -> p k h e", e=65)[:, :, :, 64:65],
                    ones_col[:, None, None, :].broadcast_to([128, NT_T, HPC, 1]),
                )

                # single sync HWDGE queue; order = need order. One coalesced
                # descriptor per logical block (SP issue is ~0.7us/descr).
                xT_r = xT.rearrange("(c p) t -> p c t", p=128)
                wqk_r = wqk.rearrange("(c p) o -> p c o", p=128)
                wv_r = wv.rearrange("(c p) o -> p c o", p=128)
                nc.sync.dma_start(out=bqk_sb[:, :],
                                  in_=bqk[:, :].rearrange("o p -> p o"))
                nc.sync.dma_start(out=wqk_sb[:, :, 0:256], in_=wqk_r[:, :, 0:256])
                # first x chunk split in two so the proj chain starts on the
                # first half while the second transfers
                nc.sync.dma_start(out=xT_sb[:, 0:4, 0:512], in_=xT_r[:, 0:4, 0:512])
                nc.sync.dma_start(out=xT_sb[:, 4:8, 0:512], in_=xT_r[:, 4:8, 0:512])
                nc.sync.dma_start(out=bv_sb[:, :], in_=bv[:, :])
                nc.sync.dma_start(out=wv_sb[:, :, :], in_=wv_r[:, :, :])
                for tt in range(1, NQ):
                    nc.sync.dma_start(
                        out=xT_sb[:, :, 512 * tt:512 * (tt + 1)],
                        in_=xT_r[:, :, 512 * tt:512 * (tt + 1)])
                nc.sync.dma_start(out=wqk_sb[:, :, 256:512], in_=wqk_r[:, :, 256:512])

                # bias row broadcast for the v projection (constant)
                nc.gpsimd.partition_broadcast(bv_bc[:, :], bv_sb[:, :], channels=128)

                # ---- pools for the whole pipeline: 2*2 + 2*2 = 8 banks ----
                with tc.tile_pool(name="ps_s", bufs=2, space="PSUM") as pss, \
                     tc.tile_pool(name="ps_o", bufs=2, space="PSUM") as pso, \
                     tc.tile_pool(name="esb", bufs=10) as esb, \
                     tc.tile_pool(name="small", bufs=4) as smb:

                    def qk_proj(ot, tt):
                        ps = pss.tile([128, 1024], F32, tag="s", name="psqk")
                        for ct in range(NT_C):
                            nc.tensor.matmul(
                                ps[:, 0:512],
                                wqk_sb[:, ct, 128 * ot:128 * (ot + 1)],
                                xT_sb[:, ct, 512 * tt:512 * (tt + 1)],
                                start=(ct == 0), stop=(ct == NT_C - 1),
                            )
                        nc.vector.tensor_scalar_add(
                            qkT[:, ot, 512 * tt:512 * (tt + 1)], ps[:, 0:512],
                            bqk_sb[:, ot:ot + 1],
                        )

                    def v_proj(tv):
                        psv = pss.tile([128, 1024], F32, tag="s", name="psv")
                        for ct in range(NT_C):
                            nc.tensor.matmul(
                                psv[:, 0:256],
                                xT_sb[:, ct, 128 * tv:128 * (tv + 1)],
                                wv_sb[:, ct, :],
                                start=(ct == 0), stop=(ct == NT_C - 1),
                            )
                        # copy + bias add in one DVE pass
                        nc.vector.scalar_tensor_tensor(
                            v_aug.rearrange("p k (h e) -> p k h e", e=65)[:, tv, :, 0:64],
                            psv[:, 0:256].rearrange("p (h e) -> p h e", e=64),
                            1.0,
                            bv_bc.rearrange("p (h e) -> p h e", e=64)[:, :, :],
                            mybir.AluOpType.mult,
                            mybir.AluOpType.add,
                        )

                    E = [None] * 128
                    po_cur = [None]

                    def s_task(i):
                        u, kt = divmod(i, NT_T)
                        pr, qt = divmod(u, NQ)
                        ps2 = pss.tile([128, 1024], F32, tag="s", name="ps2")
                        for h2 in range(2):
                            pb = 64 * h2
                            nc.tensor.matmul(
                                ps2[:, 512 * h2:512 * (h2 + 1)],
                                qkT[pb:pb + 64, OTK[pr], 128 * kt:128 * (kt + 1)],
                                qkT[pb:pb + 64, OTQ[pr], 512 * qt:512 * (qt + 1)],
                                start=True, stop=True,
                            )
                        if i >= 64 and i % 4 == 1:
                            # ScalarE is the bottleneck in the back region:
                            # offload this step's exp to DVE via the bf16
                            # Schraudolph bit trick (error cancels in softmax)
                            ei = esb.tile([128, 1024], I16, tag="e", name="ei")
                            nc.vector.tensor_scalar(
                                ei[:, :], ps2[:, :], SCH_A, SCH_B,
                                mybir.AluOpType.mult, mybir.AluOpType.add)
                            E[i] = ei
                        else:
                            e = esb.tile([128, 1024], BF16, tag="e", name="e")
                            nc.scalar.activation(e[:, :], ps2[:, :],
                                                 mybir.ActivationFunctionType.Exp)
                            E[i] = e

                    def pv_task(j):
                        u, kt = divmod(j, NT_T)
                        pr, qt = divmod(u, NQ)
                        if kt == 0:
                            po_cur[0] = [
                                pso.tile([65, 512], F32, tag=f"po{h2}", name="po")
                                for h2 in range(2)
                            ]
                        po = po_cur[0]
                        for h2 in range(2):
                            h = 2 * pr + h2
                            rhs = E[j][:, 512 * h2:512 * (h2 + 1)]
                            if rhs.dtype == I16:
                                rhs = rhs.bitcast(BF16)
                            nc.tensor.matmul(
                                po[h2][:, :],
                                v_aug[:, kt, 65 * h:65 * (h + 1)],
                                rhs,
                                start=(kt == 0), stop=(kt == NT_T - 1),
                            )
                        E[j] = None
                        if kt == NT_T - 1:
                            finalize(u, po)

                    def finalize(u, po):
                        pr, qt = divmod(u, NQ)
                        # both heads' denominator rows -> one [1, 1024] chain
                        rec = smb.tile([65, 1024], F32, tag="rec", name="rec", bufs=2)
                        for h2 in range(2):
                            nc.vector.tensor_copy(
                                rec[64:65, 512 * h2:512 * (h2 + 1)],
                                po[h2][64:65, :])
                        rb = smb.tile([64, 1024], F32, tag="rb", name="rb", bufs=2)
                        # GpSimd partition_broadcast reads zeros from a
                        # base-64 source, so first move the rows to
                        # partition 0 with an SBUF->SBUF DMA
                        nc.sync.dma_start(out=rb[0:1, :], in_=rec[64:65, :])
                        nc.gpsimd.partition_broadcast(rb[:, :], rb[0:1, :],
                                                      channels=64)
                        rbr = smb.tile([64, 1024], F32, tag="rbr", name="rbr", bufs=2)
                        nc.vector.reciprocal_approx_fast(rbr[:, :], rb[:, :])
                        for h2 in range(2):
                            h = 2 * pr + h2
                            nc.vector.tensor_tensor(
                                y_sb[:, h, 512 * qt:512 * (qt + 1)],
                                po[h2][0:64, :],
                                rbr[:, 512 * h2:512 * (h2 + 1)],
                                mybir.AluOpType.mult,
                            )
                            nc.sync.dma_start(
                                out=yT[64 * h:64 * (h + 1), 512 * qt:512 * (qt + 1)],
                                in_=y_sb[:, h, 512 * qt:512 * (qt + 1)])

                    # interleave table: proj work emitted AFTER step i
                    inter = {}
                    for tv in range(12):
                        inter.setdefault(tv, []).append(("v", tv))
                    inter.setdefault(3, []).append(("qk", OTK[0], 1))
                    inter.setdefault(7, []).append(("qk", OTK[0], 2))
                    inter.setdefault(11, []).append(("qk", OTK[0], 3))
                    for tv in range(12, 16):
                        inter.setdefault(tv, []).append(("v", tv))
                    inter.setdefault(15, []).append(("qk", OTQ[0], 1))
                    inter.setdefault(19, []).append(("qk", OTQ[0], 2))
                    inter.setdefault(23, []).append(("qk", OTQ[0], 3))
                    inter.setdefault(27, []).append(("qk", OTK[1], 0))
                    inter.setdefault(31, []).append(("qk", OTK[1], 1))
                    inter.setdefault(35, []).append(("qk", OTK[1], 2))
                    inter.setdefault(39, []).append(("qk", OTK[1], 3))
                    inter.setdefault(43, []).append(("qk", OTQ[1], 0))
                    # q-pair1 tiles 1-3 are needed only at steps 80/96/112;
                    # defer them into the ACT-bound region where PE has slack
                    inter.setdefault(71, []).append(("qk", OTQ[1], 1))
                    inter.setdefault(87, []).append(("qk", OTQ[1], 2))
                    inter.setdefault(103, []).append(("qk", OTQ[1], 3))

                    # minimal lead-in: k-pair0 and q-pair0 for the first unit
                    qk_proj(OTK[0], 0)
                    qk_proj(OTQ[0], 0)

                    for i in range(128):
                        s_task(i)
                        if i - LAG >= 0:
                            pv_task(i - LAG)
                        for item in inter.get(i, ()):
                            if item[0] == "v":
                                v_proj(item[1])
                            else:
                                qk_proj(item[1], item[2])
                    for j in range(128 - LAG, 128):
                        pv_task(j)

    nc.compile()
    return nc


def kernel(x, W_proj, b_proj):
    global _BUILT, LAST_RESULT
    x = np.ascontiguousarray(np.asarray(x, dtype=np.float32))
    W_proj = np.ascontiguousarray(np.asarray(W_proj, dtype=np.float32))
    b_proj = np.ascontiguousarray(np.asarray(b_proj, dtype=np.float32))

    if _BUILT is None:
        _BUILT = _build()
    nc = _BUILT

    in_maps = []
    for c in range(N_CORES):
        b, g = divmod(c, 4)
        hs = HPC * g                      # first global head of this core
        r0 = D * hs                       # first q row
        q_rows = W_proj[r0:r0 + 256] * 0.125
        k_rows = W_proj[C + r0:C + r0 + 256]
        v_rows = W_proj[2 * C + r0:2 * C + r0 + 256]
        wqk_rows = np.concatenate(
            [q_rows[0:128], k_rows[0:128], q_rows[128:256], k_rows[128:256]], 0)
        bq = b_proj[r0:r0 + 256] * 0.125
        bk = b_proj[C + r0:C + r0 + 256]
        bqk_rows = np.stack([bq[0:128], bk[0:128], bq[128:256], bk[128:256]])
        in_maps.append({
            "xT": np.ascontiguousarray(x[b].T).astype(NPBF16),
            "wqk": np.ascontiguousarray(wqk_rows.T).astype(NPBF16),
            "wv": np.ascontiguousarray(v_rows.T).astype(NPBF16),
            "bqk": np.ascontiguousarray(bqk_rows.astype(np.float32)),
            "bv": np.ascontiguousarray(
                b_proj[2 * C + r0:2 * C + r0 + 256].reshape(1, 256)),
        })

    LAST_RESULT = run_bass_kernel_spmd(nc, in_maps, core_ids=list(range(N_CORES)))
    out = np.empty((B, T, C), dtype=np.float32)
    for c in range(N_CORES):
        b, g = divmod(c, 4)
        # yT row r = 64*h_local + d  ->  y column 256*g + r
        out[b, :, 256 * g:256 * (g + 1)] = LAST_RESULT.results[c]["yT"].T
    return out
